# revision 9
# baseline (speedup 1.0000x reference)
"""Trainium2 Bass kernel for nn_AlignConv (rotated-anchor deformable 3x3 conv + ReLU).

Contract: kernel(**inputs) takes the FULL inputs
    x       [2, 256, 128, 128] f32
    anchors [32768, 5] f32
    weight  [256, 256, 3, 3] f32
and returns the FULL output [2, 256, 128, 128] f32, running on 8 NeuronCores.

Sharding: core i handles batch i//4, output rows [(i%4)*32, (i%4)*32+32).

Device algorithm per core (N = 4096 positions, K2 = 9 kernel points):
  bilinear(x, py, px) = S[p00] + wx*Bx[p00] + wy*By[p00] + wx*wy*Bxy[p00]
  with S/Bx/By/Bxy precomputed difference banks interleaved per pixel row
  (one 2KB row per pixel). Per (k, chunk of 1024 positions):
    1) dma_gather (HBM->SBUF, 2 SWDGE queues): points on partitions
    2) 3x scalar_tensor_tensor (DVE): bilinear combine, per-partition scalars
    3) PE transpose-mode matmuls -> PSUM fp16 -> ACT copy: channels on partitions
    4) accumulate 18 K-tile matmuls into PSUM fp32; ReLU; DMA out.
"""
import numpy as np

K2 = 9
B, C, H, W, Cout = 2, 256, 128, 128, 256
STRIDE = 8
PAD_IMG = 6
N_CORES = 8
ROWS_PER_CORE = 32
N = ROWS_PER_CORE * W            # 4096 positions per core
WP = W + 2 * PAD_IMG             # 140
SLICE_ROWS = ROWS_PER_CORE + 12  # 44
IB_ROWS = SLICE_ROWS * WP        # 6160
NT = 1024                        # positions per chunk
NCHUNK = N // NT                 # 4
Q = NT // 128                    # 8 point-groups per chunk
QTOT = N // 128                  # 32
NS = NT // 512                   # 2 psum column tiles per chunk
F16 = np.float16


# ----------------------------------------------------------------- host side

def _sample_coords(anchors_b, h_lo):
    """py, px [K2, N] absolute sample coords for output rows [h_lo, h_lo+32)."""
    anc = anchors_b.reshape(H, W, 5)[h_lo:h_lo + ROWS_PER_CORE].reshape(-1, 5)
    x_ctr, y_ctr, w, h, a = [anc[:, i].astype(np.float32) for i in range(5)]
    x_ctr, y_ctr, w, h = x_ctr / STRIDE, y_ctr / STRIDE, w / STRIDE, h / STRIDE
    cos, sin = np.cos(a), np.sin(a)
    dw, dh = w / 3.0, h / 3.0
    idx = np.arange(-1, 2, dtype=np.float32)
    yy, xx = np.meshgrid(idx, idx, indexing='ij')
    kx = xx.reshape(-1)[:, None]
    ky = yy.reshape(-1)[:, None]
    x = dw[None, :] * kx
    y = dh[None, :] * ky
    px = cos[None, :] * x - sin[None, :] * y + x_ctr[None, :]
    py = sin[None, :] * x + cos[None, :] * y + y_ctr[None, :]
    return py, px


def _build_banks(x_b, h_lo):
    """Interleaved bank rows [IB_ROWS, 4*C] f16 for one core."""
    HP = H + 2 * PAD_IMG
    xp = np.zeros((HP, WP, C), np.float32)
    xp[PAD_IMG:PAD_IMG + H, PAD_IMG:PAD_IMG + W] = np.transpose(x_b, (1, 2, 0))
    S = xp[h_lo:h_lo + SLICE_ROWS]
    Bx = np.zeros_like(S)
    Bx[:, :-1] = S[:, 1:] - S[:, :-1]
    By = np.zeros_like(S)
    By[:-1] = S[1:] - S[:-1]
    Bxy = np.zeros_like(S)
    Bxy[:-1, :-1] = S[1:, 1:] - S[1:, :-1] - S[:-1, 1:] + S[:-1, :-1]
    ib = np.stack([S, Bx, By, Bxy], axis=2)     # [44, WP, 4, C]
    return np.ascontiguousarray(ib.reshape(IB_ROWS, 4 * C).astype(F16))


def _wrap16(flat):
    """[n] -> [128, n//16] int16: index i at [i%16, i//16], replicated x8."""
    n = flat.shape[0]
    w = flat.reshape(n // 16, 16).T.astype(np.int16)     # [16, n//16]
    return np.ascontiguousarray(np.tile(w, (8, 1)))      # [128, n//16]


def _core_inputs(x, anchors, weight_r, core):
    b, blk = divmod(core, 4)
    h_lo = blk * ROWS_PER_CORE
    anchors_b = anchors.reshape(B, H * W, 5)[b]
    py, px = _sample_coords(anchors_b, h_lo)
    pyp = py + (PAD_IMG - h_lo)
    pxp = px + PAD_IMG
    y0 = np.floor(pyp)
    x0 = np.floor(pxp)
    wy = (pyp - y0).astype(np.float32)
    wx = (pxp - x0).astype(np.float32)
    y0 = y0.astype(np.int64)
    x0 = x0.astype(np.int64)
    assert y0.min() >= 0 and y0.max() <= SLICE_ROWS - 2
    assert x0.min() >= 0 and x0.max() <= WP - 2
    ridx = (y0 * WP + x0).astype(np.int16)               # [K2, N]

    # gather index tensor [K2, NCHUNK, 128, NT//16]
    idx = np.empty((K2, NCHUNK, 128, NT // 16), np.int16)
    for k in range(K2):
        for ch in range(NCHUNK):
            idx[k, ch] = _wrap16(ridx[k, ch * NT:(ch + 1) * NT])

    # STT scalars [128, K2*QTOT*3] f32: col (k*QTOT + qg)*3 + j
    scal = np.empty((128, K2 * QTOT * 3), np.float32)
    coef = np.stack([wx, wy, wx * wy], axis=-1)          # [K2, N, 3]
    coef = coef.reshape(K2, QTOT, 128, 3)
    scal[:] = np.transpose(coef, (2, 0, 1, 3)).reshape(128, K2 * QTOT * 3)

    ident = np.eye(128, dtype=F16)

    return {
        "ib": _build_banks(x[b], h_lo),
        "idx": idx,
        "scal": np.ascontiguousarray(scal),
        "wr": weight_r,
        "id128": ident,
    }


def _weight_r(weight):
    """lhsT rows: wr[(k*2+ct)*128+p, o] = weight[o, ct*128+p, k], f16 [2304, 256]."""
    w = weight.reshape(Cout, C, K2).astype(np.float32)   # [o, c, k]
    w = np.transpose(w, (2, 1, 0))                       # [k, c, o]
    return np.ascontiguousarray(w.reshape(K2 * C, Cout).astype(F16))


# --------------------------------------------------------------- bass program

_CACHE = {}


def _build_program():
    import concourse.bass as bass
    import concourse.bacc as bacc
    import concourse.tile as tile
    import concourse.mybir as mybir
    from concourse import library_config
    from contextlib import ExitStack

    f16 = mybir.dt.float16
    f32 = mybir.dt.float32
    i16 = mybir.dt.int16
    Alu = mybir.AluOpType

    nc = bacc.Bacc(None, target_bir_lowering=False, debug=False,
                   num_swdge_queues=2)
    ib = nc.dram_tensor("ib", [IB_ROWS, 4 * C], f16, kind="ExternalInput")
    idx = nc.dram_tensor("idx", [K2, NCHUNK, 128, NT // 16], i16,
                         kind="ExternalInput")
    scal = nc.dram_tensor("scal", [128, K2 * QTOT * 3], f32,
                          kind="ExternalInput")
    wr = nc.dram_tensor("wr", [K2 * C, Cout], f16, kind="ExternalInput")
    id128 = nc.dram_tensor("id128", [128, 128], f16, kind="ExternalInput")
    y = nc.dram_tensor("y", [Cout, N], f32, kind="ExternalOutput")

    with tile.TileContext(nc) as tc, ExitStack() as ctx:
        nc.gpsimd.load_library(library_config.mlp)

        const = ctx.enter_context(tc.tile_pool(name="const", bufs=1))
        ipool = ctx.enter_context(tc.tile_pool(name="idx", bufs=3))
        gpool = ctx.enter_context(tc.tile_pool(name="g", bufs=3))
        spool = ctx.enter_context(tc.tile_pool(name="s", bufs=2))
        rpool = ctx.enter_context(tc.tile_pool(name="r", bufs=2))
        opool = ctx.enter_context(tc.tile_pool(name="o", bufs=4))
        ppool = ctx.enter_context(
            tc.tile_pool(name="psum", bufs=1, space="PSUM"))
        tpool = ctx.enter_context(
            tc.tile_pool(name="tpsum", bufs=1, space="PSUM"))

        w_sb = const.tile([128, 18 * 256], f16)
        for t in range(18):
            nc.sync.dma_start(w_sb[:, t * 256:(t + 1) * 256],
                              wr[t * 128:(t + 1) * 128, :])
        scal_sb = const.tile([128, K2 * QTOT * 3], f32)
        nc.sync.dma_start(scal_sb[:], scal[:])
        id_sb = const.tile([128, 128], f16)
        nc.sync.dma_start(id_sb[:], id128[:])

        for ch in range(NCHUNK):
            psums = [[ppool.tile([128, 512], f32, name=f"ps{mt}_{ns}",
                                 tag=f"ps{mt}_{ns}")
                      for ns in range(NS)] for mt in range(2)]
            for k in range(K2):
                it = ipool.tile([128, NT // 16], i16)
                nc.sync.dma_start(it[:], idx[k, ch])

                g = gpool.tile([128, Q * 1024], f16)
                nc.gpsimd.dma_gather(
                    out_ap=g[:].rearrange("p (q e) -> p q e", e=1024),
                    in_ap=ib[:],
                    idxs_ap=it[:],
                    num_idxs=NT,
                    num_idxs_reg=NT,
                    elem_size=1024,
                    single_packet=False,
                    queue_num=(ch * K2 + k) % 2,
                )

                s = spool.tile([128, Q * 256], f16)
                for j in range(3):
                    for q in range(Q):
                        col = (k * QTOT + ch * Q + q) * 3
                        sl = s[:, q * 256:(q + 1) * 256]
                        nc.vector.scalar_tensor_tensor(
                            out=sl,
                            in0=g[:, q * 1024 + (j + 1) * 256:
                                  q * 1024 + (j + 2) * 256],
                            scalar=scal_sb[:, col + j:col + j + 1],
                            in1=g[:, q * 1024:q * 1024 + 256] if j == 0 else sl,
                            op0=Alu.mult,
                            op1=Alu.add,
                        )

                # transpose s [pts, c] -> r [c(2ct), pts] via PE transpose mode
                r = rpool.tile([128, 2 * NT], f16)
                for ct in range(2):
                    for qg in range(Q // 4):
                        pt = tpool.tile([128, 512], f16, name=f"tp{ct}_{qg}",
                                        tag=f"tp{ct}_{qg}")
                        for qi in range(4):
                            q = qg * 4 + qi
                            nc.tensor.matmul(
                                out=pt[:, qi * 128:(qi + 1) * 128],
                                lhsT=s[:, q * 256 + ct * 128:
                                       q * 256 + ct * 128 + 128],
                                rhs=id_sb[:],
                                is_transpose=True,
                            )
                        nc.scalar.activation(
                            r[:, ct * NT + qg * 512:ct * NT + (qg + 1) * 512],
                            pt[:],
                            mybir.ActivationFunctionType.Copy)

                for ct in range(2):
                    for mt in range(2):
                        lt = (k * 2 + ct) * 256 + mt * 128
                        lhsT = w_sb[:, lt:lt + 128]
                        for ns in range(NS):
                            nc.tensor.matmul(
                                out=psums[mt][ns][:],
                                lhsT=lhsT,
                                rhs=r[:, ct * NT + ns * 512:
                                      ct * NT + (ns + 1) * 512],
                                start=(k == 0 and ct == 0),
                                stop=(k == K2 - 1 and ct == 1),
                            )

            for mt in range(2):
                for ns in range(NS):
                    o = opool.tile([128, 512], f32)
                    nc.scalar.activation(
                        o[:], psums[mt][ns][:],
                        mybir.ActivationFunctionType.Relu)
                    nc.sync.dma_start(
                        y[mt * 128:(mt + 1) * 128,
                          ch * NT + ns * 512:ch * NT + (ns + 1) * 512],
                        o[:])

    nc.compile()
    return nc


def get_program():
    if "nc" not in _CACHE:
        _CACHE["nc"] = _build_program()
    return _CACHE["nc"]


def make_in_maps(x, anchors, weight):
    wr = _weight_r(weight)
    return [_core_inputs(x, anchors, wr, core) for core in range(N_CORES)]


def assemble(results):
    out = np.empty((B, Cout, H, W), np.float32)
    for core, res in enumerate(results):
        b, blk = divmod(core, 4)
        h_lo = blk * ROWS_PER_CORE
        out[b, :, h_lo:h_lo + ROWS_PER_CORE] = \
            res["y"].reshape(Cout, ROWS_PER_CORE, W)
    return out


def kernel(x, anchors, weight):
    from concourse.bass_utils import run_bass_kernel_spmd
    x = np.asarray(x, np.float32)
    anchors = np.asarray(anchors, np.float32)
    weight = np.asarray(weight, np.float32)
    nc = get_program()
    in_maps = make_in_maps(x, anchors, weight)
    res = run_bass_kernel_spmd(nc, in_maps, core_ids=list(range(N_CORES)))
    _CACHE["last_result"] = res
    return assemble(res.results)


# revision 11
# speedup vs baseline: 1.2873x; 1.2873x over previous
"""Trainium2 Bass kernel for nn_AlignConv (rotated-anchor deformable 3x3 conv + ReLU).

Contract: kernel(**inputs) takes the FULL inputs
    x       [2, 256, 128, 128] f32
    anchors [32768, 5] f32
    weight  [256, 256, 3, 3] f32
and returns the FULL output [2, 256, 128, 128] f32, running on 8 NeuronCores.

Sharding: core i handles batch i//4, output rows [(i%4)*32, (i%4)*32+32).

Device algorithm per core (N = 4096 positions, K2 = 9 kernel points):
  bilinear(x, py, px) = S[p00] + wx*Bx[p00] + wy*By[p00] + wx*wy*Bxy[p00]
  with S/Bx/By/Bxy precomputed difference banks interleaved per pixel row
  (one 2KB row per pixel). Per (k, chunk of 1024 positions):
    1) dma_gather (HBM->SBUF, 2 SWDGE queues): points on partitions
    2) 3x scalar_tensor_tensor (DVE): bilinear combine, per-partition scalars
    3) PE transpose-mode matmuls -> PSUM fp16 -> ACT copy: channels on partitions
    4) accumulate 18 K-tile matmuls into PSUM fp32; ReLU; DMA out.
"""
import numpy as np

K2 = 9
B, C, H, W, Cout = 2, 256, 128, 128, 256
STRIDE = 8
PAD_IMG = 6
N_CORES = 8
ROWS_PER_CORE = 32
N = ROWS_PER_CORE * W            # 4096 positions per core
WP = W + 2 * PAD_IMG             # 140
SLICE_ROWS = ROWS_PER_CORE + 12  # 44
IB_ROWS = SLICE_ROWS * WP        # 6160
NT = 1024                        # positions per chunk
NCHUNK = N // NT                 # 4
Q = NT // 128                    # 8 point-groups per chunk
QTOT = N // 128                  # 32
NS = NT // 512                   # 2 psum column tiles per chunk
F16 = np.float16


# ----------------------------------------------------------------- host side

def _sample_coords(anchors_b, h_lo):
    """py, px [K2, N] absolute sample coords for output rows [h_lo, h_lo+32)."""
    anc = anchors_b.reshape(H, W, 5)[h_lo:h_lo + ROWS_PER_CORE].reshape(-1, 5)
    x_ctr, y_ctr, w, h, a = [anc[:, i].astype(np.float32) for i in range(5)]
    x_ctr, y_ctr, w, h = x_ctr / STRIDE, y_ctr / STRIDE, w / STRIDE, h / STRIDE
    cos, sin = np.cos(a), np.sin(a)
    dw, dh = w / 3.0, h / 3.0
    idx = np.arange(-1, 2, dtype=np.float32)
    yy, xx = np.meshgrid(idx, idx, indexing='ij')
    kx = xx.reshape(-1)[:, None]
    ky = yy.reshape(-1)[:, None]
    x = dw[None, :] * kx
    y = dh[None, :] * ky
    px = cos[None, :] * x - sin[None, :] * y + x_ctr[None, :]
    py = sin[None, :] * x + cos[None, :] * y + y_ctr[None, :]
    return py, px


def _build_banks(x_b, h_lo):
    """Interleaved bank rows [IB_ROWS, 4*C] f16 for one core."""
    HP = H + 2 * PAD_IMG
    xp = np.zeros((HP, WP, C), np.float32)
    xp[PAD_IMG:PAD_IMG + H, PAD_IMG:PAD_IMG + W] = np.transpose(x_b, (1, 2, 0))
    S = xp[h_lo:h_lo + SLICE_ROWS]
    Bx = np.zeros_like(S)
    Bx[:, :-1] = S[:, 1:] - S[:, :-1]
    By = np.zeros_like(S)
    By[:-1] = S[1:] - S[:-1]
    Bxy = np.zeros_like(S)
    Bxy[:-1, :-1] = S[1:, 1:] - S[1:, :-1] - S[:-1, 1:] + S[:-1, :-1]
    ib = np.stack([S, Bx, By, Bxy], axis=2)     # [44, WP, 4, C]
    return np.ascontiguousarray(ib.reshape(IB_ROWS, 4 * C).astype(F16))


def _wrap16(flat):
    """[n] -> [128, n//16] int16: index i at [i%16, i//16], replicated x8."""
    n = flat.shape[0]
    w = flat.reshape(n // 16, 16).T.astype(np.int16)     # [16, n//16]
    return np.ascontiguousarray(np.tile(w, (8, 1)))      # [128, n//16]


def _core_inputs(x, anchors, weight_r, core):
    b, blk = divmod(core, 4)
    h_lo = blk * ROWS_PER_CORE
    anchors_b = anchors.reshape(B, H * W, 5)[b]
    py, px = _sample_coords(anchors_b, h_lo)
    pyp = py + (PAD_IMG - h_lo)
    pxp = px + PAD_IMG
    y0 = np.floor(pyp)
    x0 = np.floor(pxp)
    wy = (pyp - y0).astype(np.float32)
    wx = (pxp - x0).astype(np.float32)
    y0 = y0.astype(np.int64)
    x0 = x0.astype(np.int64)
    assert y0.min() >= 0 and y0.max() <= SLICE_ROWS - 2
    assert x0.min() >= 0 and x0.max() <= WP - 2
    ridx = (y0 * WP + x0).astype(np.int16)               # [K2, N]

    # gather index tensor [K2, NCHUNK, 128, NT//16]
    idx = np.empty((K2, NCHUNK, 128, NT // 16), np.int16)
    for k in range(K2):
        for ch in range(NCHUNK):
            idx[k, ch] = _wrap16(ridx[k, ch * NT:(ch + 1) * NT])

    # STT scalars [128, K2*QTOT*3] f32: col (k*QTOT + qg)*3 + j
    scal = np.empty((128, K2 * QTOT * 3), np.float32)
    coef = np.stack([wx, wy, wx * wy], axis=-1)          # [K2, N, 3]
    coef = coef.reshape(K2, QTOT, 128, 3)
    scal[:] = np.transpose(coef, (2, 0, 1, 3)).reshape(128, K2 * QTOT * 3)

    ident = np.eye(128, dtype=F16)

    return {
        "ib": _build_banks(x[b], h_lo),
        "idx": idx,
        "scal": np.ascontiguousarray(scal),
        "wr": weight_r,
        "id128": ident,
    }


def _weight_r(weight):
    """lhsT rows: wr[(k*2+ct)*128+p, o] = weight[o, ct*128+p, k], f16 [2304, 256]."""
    w = weight.reshape(Cout, C, K2).astype(np.float32)   # [o, c, k]
    w = np.transpose(w, (2, 1, 0))                       # [k, c, o]
    return np.ascontiguousarray(w.reshape(K2 * C, Cout).astype(F16))


# --------------------------------------------------------------- bass program

_CACHE = {}


def _build_program():
    import concourse.bass as bass
    import concourse.bacc as bacc
    import concourse.tile as tile
    import concourse.mybir as mybir
    from concourse import library_config
    from contextlib import ExitStack

    f16 = mybir.dt.float16
    f32 = mybir.dt.float32
    i16 = mybir.dt.int16
    Alu = mybir.AluOpType

    nc = bacc.Bacc(None, target_bir_lowering=False, debug=False,
                   num_swdge_queues=2)
    ib = nc.dram_tensor("ib", [IB_ROWS, 4 * C], f16, kind="ExternalInput")
    idx = nc.dram_tensor("idx", [K2, NCHUNK, 128, NT // 16], i16,
                         kind="ExternalInput")
    scal = nc.dram_tensor("scal", [128, K2 * QTOT * 3], f32,
                          kind="ExternalInput")
    wr = nc.dram_tensor("wr", [K2 * C, Cout], f16, kind="ExternalInput")
    id128 = nc.dram_tensor("id128", [128, 128], f16, kind="ExternalInput")
    y = nc.dram_tensor("y", [Cout, N], f32, kind="ExternalOutput")

    with tile.TileContext(nc) as tc, ExitStack() as ctx:
        nc.gpsimd.load_library(library_config.mlp)

        const = ctx.enter_context(tc.tile_pool(name="const", bufs=1))
        ipool = ctx.enter_context(tc.tile_pool(name="idx", bufs=3))
        gpool = ctx.enter_context(tc.tile_pool(name="g", bufs=2))
        spool = ctx.enter_context(tc.tile_pool(name="s", bufs=2))
        rpool = ctx.enter_context(tc.tile_pool(name="r", bufs=2))
        txpool = ctx.enter_context(tc.tile_pool(name="t2", bufs=2))
        opool = ctx.enter_context(tc.tile_pool(name="o", bufs=4))
        ppool = ctx.enter_context(
            tc.tile_pool(name="psum", bufs=1, space="PSUM"))
        tpool = ctx.enter_context(
            tc.tile_pool(name="tpsum", bufs=1, space="PSUM"))

        w_sb = const.tile([128, 18 * 256], f16)
        for t in range(18):
            nc.sync.dma_start(w_sb[:, t * 256:(t + 1) * 256],
                              wr[t * 128:(t + 1) * 128, :])
        scal_sb = const.tile([128, K2 * QTOT * 3], f32)
        nc.sync.dma_start(scal_sb[:], scal[:])
        id_sb = const.tile([128, 128], f16)
        nc.sync.dma_start(id_sb[:], id128[:])

        for ch in range(NCHUNK):
            psums = [[ppool.tile([128, 512], f32, name=f"ps{mt}_{ns}",
                                 tag=f"ps{mt}_{ns}")
                      for ns in range(NS)] for mt in range(2)]
            for kp in range(5):
                kk = [2 * kp] if kp == 4 else [2 * kp, 2 * kp + 1]
                nk = len(kk)
                npts = nk * NT
                it = ipool.tile([128, nk * NT // 16], i16, name="it", tag="it")
                for i, k in enumerate(kk):
                    nc.sync.dma_start(
                        it[:, i * (NT // 16):(i + 1) * (NT // 16)], idx[k, ch])

                g = gpool.tile([128, nk * Q * 1024], f16, name="g", tag="g")
                nc.gpsimd.dma_gather(
                    out_ap=g[:].rearrange("p (q e) -> p q e", e=1024),
                    in_ap=ib[:],
                    idxs_ap=it[:],
                    num_idxs=npts,
                    num_idxs_reg=npts,
                    elem_size=1024,
                    single_packet=False,
                    queue_num=(ch * 5 + kp) % 2,
                )

                s = spool.tile([128, nk * Q * 256], f16, name="s", tag="s")
                t2 = txpool.tile([128, nk * Q * 256], f16, name="t2", tag="t2")
                for j in range(3):
                    for qq in range(nk * Q):
                        k = kk[qq // Q]
                        col = (k * QTOT + ch * Q + qq % Q) * 3
                        sl = s[:, qq * 256:(qq + 1) * 256]
                        if j == 2:
                            t2l = t2[:, qq * 256:(qq + 1) * 256]
                            nc.scalar.activation(
                                t2l,
                                g[:, qq * 1024 + 768:qq * 1024 + 1024],
                                mybir.ActivationFunctionType.Copy,
                                scale=scal_sb[:, col + 2:col + 3])
                            nc.vector.tensor_tensor(
                                out=sl, in0=sl, in1=t2l, op=Alu.add)
                        else:
                            nc.vector.scalar_tensor_tensor(
                                out=sl,
                                in0=g[:, qq * 1024 + (j + 1) * 256:
                                      qq * 1024 + (j + 2) * 256],
                                scalar=scal_sb[:, col + j:col + j + 1],
                                in1=(g[:, qq * 1024:qq * 1024 + 256]
                                     if j == 0 else sl),
                                op0=Alu.mult,
                                op1=Alu.add,
                            )

                for ki, k in enumerate(kk):
                    # transpose s [pts, c] -> r [c(2ct), pts], PE transpose mode
                    r = rpool.tile([128, 2 * NT], f16, name="r", tag="r")
                    for ct in range(2):
                        for qg in range(Q // 4):
                            pt = tpool.tile([128, 512], f16,
                                            name=f"tp{ct}_{qg}",
                                            tag=f"tp{ct}_{qg}")
                            for qi in range(4):
                                q = ki * Q + qg * 4 + qi
                                nc.tensor.matmul(
                                    out=pt[:, qi * 128:(qi + 1) * 128],
                                    lhsT=s[:, q * 256 + ct * 128:
                                           q * 256 + ct * 128 + 128],
                                    rhs=id_sb[:],
                                    is_transpose=True,
                                )
                            nc.scalar.activation(
                                r[:, ct * NT + qg * 512:
                                  ct * NT + (qg + 1) * 512],
                                pt[:],
                                mybir.ActivationFunctionType.Copy)

                    for ct in range(2):
                        for mt in range(2):
                            lt = (k * 2 + ct) * 256 + mt * 128
                            lhsT = w_sb[:, lt:lt + 128]
                            for ns in range(NS):
                                nc.tensor.matmul(
                                    out=psums[mt][ns][:],
                                    lhsT=lhsT,
                                    rhs=r[:, ct * NT + ns * 512:
                                          ct * NT + (ns + 1) * 512],
                                    start=(k == 0 and ct == 0),
                                    stop=(k == K2 - 1 and ct == 1),
                                )

            for mt in range(2):
                for ns in range(NS):
                    o = opool.tile([128, 512], f32)
                    nc.scalar.activation(
                        o[:], psums[mt][ns][:],
                        mybir.ActivationFunctionType.Relu)
                    nc.sync.dma_start(
                        y[mt * 128:(mt + 1) * 128,
                          ch * NT + ns * 512:ch * NT + (ns + 1) * 512],
                        o[:])

    nc.compile()
    return nc


def get_program():
    if "nc" not in _CACHE:
        _CACHE["nc"] = _build_program()
    return _CACHE["nc"]


def make_in_maps(x, anchors, weight):
    wr = _weight_r(weight)
    return [_core_inputs(x, anchors, wr, core) for core in range(N_CORES)]


def assemble(results):
    out = np.empty((B, Cout, H, W), np.float32)
    for core, res in enumerate(results):
        b, blk = divmod(core, 4)
        h_lo = blk * ROWS_PER_CORE
        out[b, :, h_lo:h_lo + ROWS_PER_CORE] = \
            res["y"].reshape(Cout, ROWS_PER_CORE, W)
    return out


def kernel(x, anchors, weight):
    from concourse.bass_utils import run_bass_kernel_spmd
    x = np.asarray(x, np.float32)
    anchors = np.asarray(anchors, np.float32)
    weight = np.asarray(weight, np.float32)
    nc = get_program()
    in_maps = make_in_maps(x, anchors, weight)
    res = run_bass_kernel_spmd(nc, in_maps, core_ids=list(range(N_CORES)))
    _CACHE["last_result"] = res
    return assemble(res.results)


# revision 12
# speedup vs baseline: 1.3182x; 1.0240x over previous
"""Trainium2 Bass kernel for nn_AlignConv (rotated-anchor deformable 3x3 conv + ReLU).

Contract: kernel(**inputs) takes the FULL inputs
    x       [2, 256, 128, 128] f32
    anchors [32768, 5] f32
    weight  [256, 256, 3, 3] f32
and returns the FULL output [2, 256, 128, 128] f32, running on 8 NeuronCores.

Sharding: core i handles batch i//4, output rows [(i%4)*32, (i%4)*32+32).

Device algorithm per core (N = 4096 positions, K2 = 9 kernel points):
  bilinear(x, py, px) = S[p00] + wx*Bx[p00] + wy*By[p00] + wx*wy*Bxy[p00]
  with S/Bx/By/Bxy precomputed difference banks interleaved per pixel row
  (one 2KB row per pixel). Per (k, chunk of 1024 positions):
    1) dma_gather (HBM->SBUF, 2 SWDGE queues): points on partitions
    2) 3x scalar_tensor_tensor (DVE): bilinear combine, per-partition scalars
    3) PE transpose-mode matmuls -> PSUM fp16 -> ACT copy: channels on partitions
    4) accumulate 18 K-tile matmuls into PSUM fp32; ReLU; DMA out.
"""
import numpy as np

K2 = 9
B, C, H, W, Cout = 2, 256, 128, 128, 256
STRIDE = 8
PAD_IMG = 6
N_CORES = 8
ROWS_PER_CORE = 32
N = ROWS_PER_CORE * W            # 4096 positions per core
WP = W + 2 * PAD_IMG             # 140
SLICE_ROWS = ROWS_PER_CORE + 12  # 44
IB_ROWS = SLICE_ROWS * WP        # 6160
NT = 1024                        # positions per chunk
NCHUNK = N // NT                 # 4
Q = NT // 128                    # 8 point-groups per chunk
QTOT = N // 128                  # 32
NS = NT // 512                   # 2 psum column tiles per chunk
F16 = np.float16


# ----------------------------------------------------------------- host side

def _sample_coords(anchors_b, h_lo):
    """py, px [K2, N] absolute sample coords for output rows [h_lo, h_lo+32)."""
    anc = anchors_b.reshape(H, W, 5)[h_lo:h_lo + ROWS_PER_CORE].reshape(-1, 5)
    x_ctr, y_ctr, w, h, a = [anc[:, i].astype(np.float32) for i in range(5)]
    x_ctr, y_ctr, w, h = x_ctr / STRIDE, y_ctr / STRIDE, w / STRIDE, h / STRIDE
    cos, sin = np.cos(a), np.sin(a)
    dw, dh = w / 3.0, h / 3.0
    idx = np.arange(-1, 2, dtype=np.float32)
    yy, xx = np.meshgrid(idx, idx, indexing='ij')
    kx = xx.reshape(-1)[:, None]
    ky = yy.reshape(-1)[:, None]
    x = dw[None, :] * kx
    y = dh[None, :] * ky
    px = cos[None, :] * x - sin[None, :] * y + x_ctr[None, :]
    py = sin[None, :] * x + cos[None, :] * y + y_ctr[None, :]
    return py, px


def _build_banks(x_b, h_lo):
    """Interleaved bank rows [IB_ROWS, 4*C] f16 for one core."""
    HP = H + 2 * PAD_IMG
    xp = np.zeros((HP, WP, C), np.float32)
    xp[PAD_IMG:PAD_IMG + H, PAD_IMG:PAD_IMG + W] = np.transpose(x_b, (1, 2, 0))
    S = xp[h_lo:h_lo + SLICE_ROWS]
    Bx = np.zeros_like(S)
    Bx[:, :-1] = S[:, 1:] - S[:, :-1]
    By = np.zeros_like(S)
    By[:-1] = S[1:] - S[:-1]
    Bxy = np.zeros_like(S)
    Bxy[:-1, :-1] = S[1:, 1:] - S[1:, :-1] - S[:-1, 1:] + S[:-1, :-1]
    ib = np.stack([S, Bx, By, Bxy], axis=2)     # [44, WP, 4, C]
    return np.ascontiguousarray(ib.reshape(IB_ROWS, 4 * C).astype(F16))


def _wrap16(flat):
    """[n] -> [128, n//16] int16: index i at [i%16, i//16], replicated x8."""
    n = flat.shape[0]
    w = flat.reshape(n // 16, 16).T.astype(np.int16)     # [16, n//16]
    return np.ascontiguousarray(np.tile(w, (8, 1)))      # [128, n//16]


def _core_inputs(x, anchors, weight_r, core):
    b, blk = divmod(core, 4)
    h_lo = blk * ROWS_PER_CORE
    anchors_b = anchors.reshape(B, H * W, 5)[b]
    py, px = _sample_coords(anchors_b, h_lo)
    pyp = py + (PAD_IMG - h_lo)
    pxp = px + PAD_IMG
    y0 = np.floor(pyp)
    x0 = np.floor(pxp)
    wy = (pyp - y0).astype(np.float32)
    wx = (pxp - x0).astype(np.float32)
    y0 = y0.astype(np.int64)
    x0 = x0.astype(np.int64)
    assert y0.min() >= 0 and y0.max() <= SLICE_ROWS - 2
    assert x0.min() >= 0 and x0.max() <= WP - 2
    ridx = (y0 * WP + x0).astype(np.int16)               # [K2, N]

    # gather index tensor [K2, NCHUNK, 128, NT//16]
    idx = np.empty((K2, NCHUNK, 128, NT // 16), np.int16)
    for k in range(K2):
        for ch in range(NCHUNK):
            idx[k, ch] = _wrap16(ridx[k, ch * NT:(ch + 1) * NT])

    # STT scalars [128, K2*QTOT*3] f32: col (k*QTOT + qg)*3 + j
    scal = np.empty((128, K2 * QTOT * 3), np.float32)
    coef = np.stack([wx, wy, wx * wy], axis=-1)          # [K2, N, 3]
    coef = coef.reshape(K2, QTOT, 128, 3)
    scal[:] = np.transpose(coef, (2, 0, 1, 3)).reshape(128, K2 * QTOT * 3)

    ident = np.eye(128, dtype=F16)

    return {
        "ib": _build_banks(x[b], h_lo),
        "idx": idx,
        "scal": np.ascontiguousarray(scal),
        "wr": weight_r,
        "id128": ident,
    }


def _weight_r(weight):
    """lhsT rows: wr[(k*2+ct)*128+p, o] = weight[o, ct*128+p, k], f16 [2304, 256]."""
    w = weight.reshape(Cout, C, K2).astype(np.float32)   # [o, c, k]
    w = np.transpose(w, (2, 1, 0))                       # [k, c, o]
    return np.ascontiguousarray(w.reshape(K2 * C, Cout).astype(F16))


# --------------------------------------------------------------- bass program

_CACHE = {}


def _build_program():
    import concourse.bass as bass
    import concourse.bacc as bacc
    import concourse.tile as tile
    import concourse.mybir as mybir
    from concourse import library_config
    from contextlib import ExitStack

    f16 = mybir.dt.float16
    f32 = mybir.dt.float32
    i16 = mybir.dt.int16
    Alu = mybir.AluOpType

    nc = bacc.Bacc(None, target_bir_lowering=False, debug=False,
                   num_swdge_queues=2)
    ib = nc.dram_tensor("ib", [IB_ROWS, 4 * C], f16, kind="ExternalInput")
    idx = nc.dram_tensor("idx", [K2, NCHUNK, 128, NT // 16], i16,
                         kind="ExternalInput")
    scal = nc.dram_tensor("scal", [128, K2 * QTOT * 3], f32,
                          kind="ExternalInput")
    wr = nc.dram_tensor("wr", [K2 * C, Cout], f16, kind="ExternalInput")
    id128 = nc.dram_tensor("id128", [128, 128], f16, kind="ExternalInput")
    y = nc.dram_tensor("y", [Cout, N], f32, kind="ExternalOutput")

    with tile.TileContext(nc) as tc, ExitStack() as ctx:
        nc.gpsimd.load_library(library_config.mlp)

        const = ctx.enter_context(tc.tile_pool(name="const", bufs=1))
        ipool = ctx.enter_context(tc.tile_pool(name="idx", bufs=3))
        gpool = ctx.enter_context(tc.tile_pool(name="g", bufs=3))
        spool = ctx.enter_context(tc.tile_pool(name="s", bufs=2))
        rpool = ctx.enter_context(tc.tile_pool(name="r", bufs=2))
        dpool = ctx.enter_context(tc.tile_pool(name="dg", bufs=2))
        opool = ctx.enter_context(tc.tile_pool(name="o", bufs=4))
        ppool = ctx.enter_context(
            tc.tile_pool(name="psum", bufs=1, space="PSUM"))
        tpool = ctx.enter_context(
            tc.tile_pool(name="tpsum", bufs=1, space="PSUM"))

        w_sb = const.tile([128, 18 * 256], f16)
        for t in range(18):
            nc.sync.dma_start(w_sb[:, t * 256:(t + 1) * 256],
                              wr[t * 128:(t + 1) * 128, :])
        scal_sb = const.tile([128, K2 * QTOT * 3], f32)
        nc.sync.dma_start(scal_sb[:], scal[:])
        id_sb = const.tile([128, 128], f16)
        nc.sync.dma_start(id_sb[:], id128[:])

        for ch in range(NCHUNK):
            psums = [[ppool.tile([128, 512], f32, name=f"ps{mt}_{ns}",
                                 tag=f"ps{mt}_{ns}")
                      for ns in range(NS)] for mt in range(2)]
            for kp in range(5):
                kk = [2 * kp] if kp == 4 else [2 * kp, 2 * kp + 1]
                nk = len(kk)
                npts = nk * NT
                it = ipool.tile([128, nk * NT // 16], i16, name="it", tag="it")
                for i, k in enumerate(kk):
                    nc.sync.dma_start(
                        it[:, i * (NT // 16):(i + 1) * (NT // 16)], idx[k, ch])

                g = gpool.tile([128, nk * Q * 1024], f16, name="g", tag="g")
                nc.gpsimd.dma_gather(
                    out_ap=g[:].rearrange("p (q e) -> p q e", e=1024),
                    in_ap=ib[:],
                    idxs_ap=it[:],
                    num_idxs=npts,
                    num_idxs_reg=npts,
                    elem_size=1024,
                    single_packet=False,
                    queue_num=(ch * 5 + kp) % 2,
                )

                # s01 = S + wx*Bx (one fused DVE op per point-group);
                # diag tiles D2 = diag(wy), D3 = diag(wx*wy)
                s = spool.tile([128, nk * Q * 256], f16, name="s", tag="s")
                dg = dpool.tile([128, nk * Q * 256], f16, name="dg", tag="dg")
                for qq in range(nk * Q):
                    k = kk[qq // Q]
                    col = (k * QTOT + ch * Q + qq % Q) * 3
                    nc.vector.scalar_tensor_tensor(
                        out=s[:, qq * 256:(qq + 1) * 256],
                        in0=g[:, qq * 1024 + 256:qq * 1024 + 512],
                        scalar=scal_sb[:, col:col + 1],
                        in1=g[:, qq * 1024:qq * 1024 + 256],
                        op0=Alu.mult,
                        op1=Alu.add,
                    )
                    for m in (1, 2):
                        nc.vector.tensor_scalar_mul(
                            dg[:, qq * 256 + (m - 1) * 128:
                               qq * 256 + m * 128],
                            id_sb[:],
                            scal_sb[:, col + m:col + m + 1],
                        )

                for ki, k in enumerate(kk):
                    # scaled transposes: psum[c, pts] = s01.T + By.T@D2 + Bxy.T@D3
                    r = rpool.tile([128, 2 * NT], f16, name="r", tag="r")
                    for ct in range(2):
                        for qg in range(Q // 4):
                            pt = tpool.tile([128, 512], f32,
                                            name=f"tp{ct}_{qg}",
                                            tag=f"tp{ct}_{qg}")
                            for qi in range(4):
                                q = ki * Q + qg * 4 + qi
                                po = pt[:, qi * 128:(qi + 1) * 128]
                                nc.tensor.matmul(
                                    out=po,
                                    lhsT=s[:, q * 256 + ct * 128:
                                           q * 256 + ct * 128 + 128],
                                    rhs=id_sb[:],
                                    start=True, stop=False,
                                )
                                nc.tensor.matmul(
                                    out=po,
                                    lhsT=g[:, q * 1024 + 512 + ct * 128:
                                           q * 1024 + 512 + ct * 128 + 128],
                                    rhs=dg[:, q * 256:q * 256 + 128],
                                    start=False, stop=False,
                                )
                                nc.tensor.matmul(
                                    out=po,
                                    lhsT=g[:, q * 1024 + 768 + ct * 128:
                                           q * 1024 + 768 + ct * 128 + 128],
                                    rhs=dg[:, q * 256 + 128:q * 256 + 256],
                                    start=False, stop=True,
                                )
                            nc.scalar.activation(
                                r[:, ct * NT + qg * 512:
                                  ct * NT + (qg + 1) * 512],
                                pt[:],
                                mybir.ActivationFunctionType.Copy)

                    for ct in range(2):
                        for mt in range(2):
                            lt = (k * 2 + ct) * 256 + mt * 128
                            lhsT = w_sb[:, lt:lt + 128]
                            for ns in range(NS):
                                nc.tensor.matmul(
                                    out=psums[mt][ns][:],
                                    lhsT=lhsT,
                                    rhs=r[:, ct * NT + ns * 512:
                                          ct * NT + (ns + 1) * 512],
                                    start=(k == 0 and ct == 0),
                                    stop=(k == K2 - 1 and ct == 1),
                                )

            for mt in range(2):
                for ns in range(NS):
                    o = opool.tile([128, 512], f32)
                    nc.scalar.activation(
                        o[:], psums[mt][ns][:],
                        mybir.ActivationFunctionType.Relu)
                    nc.sync.dma_start(
                        y[mt * 128:(mt + 1) * 128,
                          ch * NT + ns * 512:ch * NT + (ns + 1) * 512],
                        o[:])

    nc.compile()
    return nc


def get_program():
    if "nc" not in _CACHE:
        _CACHE["nc"] = _build_program()
    return _CACHE["nc"]


def make_in_maps(x, anchors, weight):
    wr = _weight_r(weight)
    return [_core_inputs(x, anchors, wr, core) for core in range(N_CORES)]


def assemble(results):
    out = np.empty((B, Cout, H, W), np.float32)
    for core, res in enumerate(results):
        b, blk = divmod(core, 4)
        h_lo = blk * ROWS_PER_CORE
        out[b, :, h_lo:h_lo + ROWS_PER_CORE] = \
            res["y"].reshape(Cout, ROWS_PER_CORE, W)
    return out


def kernel(x, anchors, weight):
    from concourse.bass_utils import run_bass_kernel_spmd
    x = np.asarray(x, np.float32)
    anchors = np.asarray(anchors, np.float32)
    weight = np.asarray(weight, np.float32)
    nc = get_program()
    in_maps = make_in_maps(x, anchors, weight)
    res = run_bass_kernel_spmd(nc, in_maps, core_ids=list(range(N_CORES)))
    _CACHE["last_result"] = res
    return assemble(res.results)


# revision 13
# speedup vs baseline: 1.5513x; 1.1769x over previous
"""Trainium2 Bass kernel for nn_AlignConv (rotated-anchor deformable 3x3 conv + ReLU).

Contract: kernel(**inputs) takes the FULL inputs
    x       [2, 256, 128, 128] f32
    anchors [32768, 5] f32
    weight  [256, 256, 3, 3] f32
and returns the FULL output [2, 256, 128, 128] f32, running on 8 NeuronCores.

Sharding: core i handles batch i//4, output rows [(i%4)*32, (i%4)*32+32).

Device algorithm per core (N = 4096 positions, K2 = 9 kernel points):
  bilinear(x, py, px) = S[p00] + wx*Bx[p00] + wy*By[p00] + wx*wy*Bxy[p00]
  with S/Bx/By/Bxy precomputed difference banks interleaved per pixel row
  (one 2KB row per pixel). Per (k, chunk of 1024 positions):
    1) dma_gather (HBM->SBUF, 2 SWDGE queues): points on partitions
    2) 3x scalar_tensor_tensor (DVE): bilinear combine, per-partition scalars
    3) PE transpose-mode matmuls -> PSUM fp16 -> ACT copy: channels on partitions
    4) accumulate 18 K-tile matmuls into PSUM fp32; ReLU; DMA out.
"""
import numpy as np

K2 = 9
B, C, H, W, Cout = 2, 256, 128, 128, 256
STRIDE = 8
PAD_IMG = 6
N_CORES = 8
ROWS_PER_CORE = 32
N = ROWS_PER_CORE * W            # 4096 positions per core
WP = W + 2 * PAD_IMG             # 140
SLICE_ROWS = ROWS_PER_CORE + 12  # 44
IB_ROWS = SLICE_ROWS * WP        # 6160
NT = 1024                        # positions per chunk
NCHUNK = N // NT                 # 4
Q = NT // 128                    # 8 point-groups per chunk
QTOT = N // 128                  # 32
NS = NT // 512                   # 2 psum column tiles per chunk
F16 = np.float16


# ----------------------------------------------------------------- host side

def _sample_coords(anchors_b, h_lo):
    """py, px [K2, N] absolute sample coords for output rows [h_lo, h_lo+32)."""
    anc = anchors_b.reshape(H, W, 5)[h_lo:h_lo + ROWS_PER_CORE].reshape(-1, 5)
    x_ctr, y_ctr, w, h, a = [anc[:, i].astype(np.float32) for i in range(5)]
    x_ctr, y_ctr, w, h = x_ctr / STRIDE, y_ctr / STRIDE, w / STRIDE, h / STRIDE
    cos, sin = np.cos(a), np.sin(a)
    dw, dh = w / 3.0, h / 3.0
    idx = np.arange(-1, 2, dtype=np.float32)
    yy, xx = np.meshgrid(idx, idx, indexing='ij')
    kx = xx.reshape(-1)[:, None]
    ky = yy.reshape(-1)[:, None]
    x = dw[None, :] * kx
    y = dh[None, :] * ky
    px = cos[None, :] * x - sin[None, :] * y + x_ctr[None, :]
    py = sin[None, :] * x + cos[None, :] * y + y_ctr[None, :]
    return py, px


def _build_banks(x_b, h_lo):
    """Interleaved bank rows [IB_ROWS, 4*C] f16 for one core."""
    HP = H + 2 * PAD_IMG
    xp = np.zeros((HP, WP, C), np.float32)
    xp[PAD_IMG:PAD_IMG + H, PAD_IMG:PAD_IMG + W] = np.transpose(x_b, (1, 2, 0))
    S = xp[h_lo:h_lo + SLICE_ROWS]
    Bx = np.zeros_like(S)
    Bx[:, :-1] = S[:, 1:] - S[:, :-1]
    By = np.zeros_like(S)
    By[:-1] = S[1:] - S[:-1]
    Bxy = np.zeros_like(S)
    Bxy[:-1, :-1] = S[1:, 1:] - S[1:, :-1] - S[:-1, 1:] + S[:-1, :-1]
    ib = np.stack([S, Bx, By, Bxy], axis=2)     # [44, WP, 4, C]
    return np.ascontiguousarray(ib.reshape(IB_ROWS, 4 * C).astype(F16))


def _wrap16(flat):
    """[n] -> [128, n//16] int16: index i at [i%16, i//16], replicated x8."""
    n = flat.shape[0]
    w = flat.reshape(n // 16, 16).T.astype(np.int16)     # [16, n//16]
    return np.ascontiguousarray(np.tile(w, (8, 1)))      # [128, n//16]


def _core_inputs(x, anchors, weight_r, core):
    b, blk = divmod(core, 4)
    h_lo = blk * ROWS_PER_CORE
    anchors_b = anchors.reshape(B, H * W, 5)[b]
    py, px = _sample_coords(anchors_b, h_lo)
    pyp = py + (PAD_IMG - h_lo)
    pxp = px + PAD_IMG
    y0 = np.floor(pyp)
    x0 = np.floor(pxp)
    wy = (pyp - y0).astype(np.float32)
    wx = (pxp - x0).astype(np.float32)
    y0 = y0.astype(np.int64)
    x0 = x0.astype(np.int64)
    assert y0.min() >= 0 and y0.max() <= SLICE_ROWS - 2
    assert x0.min() >= 0 and x0.max() <= WP - 2
    ridx = (y0 * WP + x0).astype(np.int16)               # [K2, N]

    # gather index tensor [K2, NCHUNK, 128, NT//16]
    idx = np.empty((K2, NCHUNK, 128, NT // 16), np.int16)
    for k in range(K2):
        for ch in range(NCHUNK):
            idx[k, ch] = _wrap16(ridx[k, ch * NT:(ch + 1) * NT])

    # STT scalars [128, K2*QTOT*3] f32: col (k*QTOT + qg)*3 + j
    scal = np.empty((128, K2 * QTOT * 3), np.float32)
    coef = np.stack([wx, wy, wx * wy], axis=-1)          # [K2, N, 3]
    coef = coef.reshape(K2, QTOT, 128, 3)
    scal[:] = np.transpose(coef, (2, 0, 1, 3)).reshape(128, K2 * QTOT * 3)

    ident = np.eye(128, dtype=F16)

    # host-built diag(wy) tiles: dgh[ch, k, p, q*128+f] = wy at point
    # (k, ch*NT + q*128 + p) when p == f else 0
    dgh = np.zeros((NCHUNK, K2, 128, Q * 128), F16)
    wyr = wy.reshape(K2, NCHUNK, Q, 128).transpose(1, 0, 2, 3)  # [ch,k,q,p]
    p = np.arange(128)
    for q in range(Q):
        dgh[:, :, p, q * 128 + p] = wyr[:, :, q, :]

    return {
        "ib": _build_banks(x[b], h_lo),
        "idx": idx,
        "scal": np.ascontiguousarray(scal),
        "wr": weight_r,
        "id128": ident,
        "dgh": dgh,
    }


def _weight_r(weight):
    """lhsT rows: wr[(k*2+ct)*128+p, o] = weight[o, ct*128+p, k], f16 [2304, 256]."""
    w = weight.reshape(Cout, C, K2).astype(np.float32)   # [o, c, k]
    w = np.transpose(w, (2, 1, 0))                       # [k, c, o]
    return np.ascontiguousarray(w.reshape(K2 * C, Cout).astype(F16))


# --------------------------------------------------------------- bass program

_CACHE = {}


def _build_program():
    import concourse.bass as bass
    import concourse.bacc as bacc
    import concourse.tile as tile
    import concourse.mybir as mybir
    from concourse import library_config
    from contextlib import ExitStack

    f16 = mybir.dt.float16
    f32 = mybir.dt.float32
    i16 = mybir.dt.int16
    Alu = mybir.AluOpType

    nc = bacc.Bacc(None, target_bir_lowering=False, debug=False,
                   num_swdge_queues=2)
    ib = nc.dram_tensor("ib", [IB_ROWS, 4 * C], f16, kind="ExternalInput")
    idx = nc.dram_tensor("idx", [K2, NCHUNK, 128, NT // 16], i16,
                         kind="ExternalInput")
    scal = nc.dram_tensor("scal", [128, K2 * QTOT * 3], f32,
                          kind="ExternalInput")
    wr = nc.dram_tensor("wr", [K2 * C, Cout], f16, kind="ExternalInput")
    id128 = nc.dram_tensor("id128", [128, 128], f16, kind="ExternalInput")
    dgh = nc.dram_tensor("dgh", [NCHUNK, K2, 128, Q * 128], f16,
                         kind="ExternalInput")
    y = nc.dram_tensor("y", [Cout, N], f32, kind="ExternalOutput")

    with tile.TileContext(nc) as tc, ExitStack() as ctx:
        nc.gpsimd.load_library(library_config.mlp)

        const = ctx.enter_context(tc.tile_pool(name="const", bufs=1))
        ipool = ctx.enter_context(tc.tile_pool(name="idx", bufs=3))
        gpool = ctx.enter_context(tc.tile_pool(name="g", bufs=3))
        spool = ctx.enter_context(tc.tile_pool(name="s", bufs=2))
        rpool = ctx.enter_context(tc.tile_pool(name="r", bufs=2))
        dpool = ctx.enter_context(tc.tile_pool(name="dg", bufs=2))
        opool = ctx.enter_context(tc.tile_pool(name="o", bufs=4))
        ppool = ctx.enter_context(
            tc.tile_pool(name="psum", bufs=1, space="PSUM"))
        tpool = ctx.enter_context(
            tc.tile_pool(name="tpsum", bufs=1, space="PSUM"))

        w_sb = const.tile([128, 18 * 256], f16)
        for t in range(18):
            nc.sync.dma_start(w_sb[:, t * 256:(t + 1) * 256],
                              wr[t * 128:(t + 1) * 128, :])
        scal_sb = const.tile([128, K2 * QTOT * 3], f32)
        nc.sync.dma_start(scal_sb[:], scal[:])
        id_sb = const.tile([128, 128], f16)
        nc.sync.dma_start(id_sb[:], id128[:])

        for ch in range(NCHUNK):
            psums = [[ppool.tile([128, 512], f32, name=f"ps{mt}_{ns}",
                                 tag=f"ps{mt}_{ns}")
                      for ns in range(NS)] for mt in range(2)]
            for kp in range(5):
                kk = [2 * kp] if kp == 4 else [2 * kp, 2 * kp + 1]
                nk = len(kk)
                npts = nk * NT
                it = ipool.tile([128, nk * NT // 16], i16, name="it", tag="it")
                for i, k in enumerate(kk):
                    nc.sync.dma_start(
                        it[:, i * (NT // 16):(i + 1) * (NT // 16)], idx[k, ch])

                g = gpool.tile([128, nk * Q * 1024], f16, name="g", tag="g")
                nc.gpsimd.dma_gather(
                    out_ap=g[:].rearrange("p (q e) -> p q e", e=1024),
                    in_ap=ib[:],
                    idxs_ap=it[:],
                    num_idxs=npts,
                    num_idxs_reg=npts,
                    elem_size=1024,
                    single_packet=False,
                    queue_num=(ch * 5 + kp) % 2,
                )

                # s01 = S + wx*Bx ; s2 = By + wx*Bxy  (2 fused DVE ops per
                # point-group); sampled = s01 + wy*s2 folded into the PE
                # transpose via diag(wy) from host
                s = spool.tile([128, nk * Q * 512], f16, name="s", tag="s")
                dg = dpool.tile([128, nk * Q * 128], f16, name="dg", tag="dg")
                for i, k in enumerate(kk):
                    nc.sync.dma_start(
                        dg[:, i * Q * 128:(i + 1) * Q * 128], dgh[ch, k])
                for qq in range(nk * Q):
                    k = kk[qq // Q]
                    col = (k * QTOT + ch * Q + qq % Q) * 3
                    nc.vector.scalar_tensor_tensor(
                        out=s[:, qq * 512:qq * 512 + 256],
                        in0=g[:, qq * 1024 + 256:qq * 1024 + 512],
                        scalar=scal_sb[:, col:col + 1],
                        in1=g[:, qq * 1024:qq * 1024 + 256],
                        op0=Alu.mult,
                        op1=Alu.add,
                    )
                    nc.vector.scalar_tensor_tensor(
                        out=s[:, qq * 512 + 256:qq * 512 + 512],
                        in0=g[:, qq * 1024 + 768:qq * 1024 + 1024],
                        scalar=scal_sb[:, col:col + 1],
                        in1=g[:, qq * 1024 + 512:qq * 1024 + 768],
                        op0=Alu.mult,
                        op1=Alu.add,
                    )

                for ki, k in enumerate(kk):
                    # scaled transposes: psum[c, pts] = s01.T + By.T@D2 + Bxy.T@D3
                    r = rpool.tile([128, 2 * NT], f16, name="r", tag="r")
                    for ct in range(2):
                        for qg in range(Q // 4):
                            pt = tpool.tile([128, 512], f32,
                                            name=f"tp{ct}_{qg}",
                                            tag=f"tp{ct}_{qg}")
                            for qi in range(4):
                                q = ki * Q + qg * 4 + qi
                                po = pt[:, qi * 128:(qi + 1) * 128]
                                nc.tensor.matmul(
                                    out=po,
                                    lhsT=s[:, q * 512 + ct * 128:
                                           q * 512 + ct * 128 + 128],
                                    rhs=id_sb[:],
                                    start=True, stop=False,
                                )
                                nc.tensor.matmul(
                                    out=po,
                                    lhsT=s[:, q * 512 + 256 + ct * 128:
                                           q * 512 + 256 + ct * 128 + 128],
                                    rhs=dg[:, q * 128:q * 128 + 128],
                                    start=False, stop=True,
                                )
                            nc.scalar.activation(
                                r[:, ct * NT + qg * 512:
                                  ct * NT + (qg + 1) * 512],
                                pt[:],
                                mybir.ActivationFunctionType.Copy)

                    for ct in range(2):
                        for mt in range(2):
                            lt = (k * 2 + ct) * 256 + mt * 128
                            lhsT = w_sb[:, lt:lt + 128]
                            for ns in range(NS):
                                nc.tensor.matmul(
                                    out=psums[mt][ns][:],
                                    lhsT=lhsT,
                                    rhs=r[:, ct * NT + ns * 512:
                                          ct * NT + (ns + 1) * 512],
                                    start=(k == 0 and ct == 0),
                                    stop=(k == K2 - 1 and ct == 1),
                                )

            for mt in range(2):
                for ns in range(NS):
                    o = opool.tile([128, 512], f32)
                    nc.scalar.activation(
                        o[:], psums[mt][ns][:],
                        mybir.ActivationFunctionType.Relu)
                    nc.sync.dma_start(
                        y[mt * 128:(mt + 1) * 128,
                          ch * NT + ns * 512:ch * NT + (ns + 1) * 512],
                        o[:])

    nc.compile()
    return nc


def get_program():
    if "nc" not in _CACHE:
        _CACHE["nc"] = _build_program()
    return _CACHE["nc"]


def make_in_maps(x, anchors, weight):
    wr = _weight_r(weight)
    return [_core_inputs(x, anchors, wr, core) for core in range(N_CORES)]


def assemble(results):
    out = np.empty((B, Cout, H, W), np.float32)
    for core, res in enumerate(results):
        b, blk = divmod(core, 4)
        h_lo = blk * ROWS_PER_CORE
        out[b, :, h_lo:h_lo + ROWS_PER_CORE] = \
            res["y"].reshape(Cout, ROWS_PER_CORE, W)
    return out


def kernel(x, anchors, weight):
    from concourse.bass_utils import run_bass_kernel_spmd
    x = np.asarray(x, np.float32)
    anchors = np.asarray(anchors, np.float32)
    weight = np.asarray(weight, np.float32)
    nc = get_program()
    in_maps = make_in_maps(x, anchors, weight)
    res = run_bass_kernel_spmd(nc, in_maps, core_ids=list(range(N_CORES)))
    _CACHE["last_result"] = res
    return assemble(res.results)
